# revision 1
# baseline (speedup 1.0000x reference)
"""Causal self-attention (B=2, T=2048, C=2048, H=16, rope) on 8 trn2 cores.

Sharding: tensor-parallel over heads. Each core owns 2 of 16 heads:
  - Wqkv columns for its heads (q,k,v), Wproj rows for its heads.
  - Computes qkv projection, rope, causal attention, and its partial
    output projection y_c = O_c @ Wproj_c  (full [4096, 2048]).
  - Host sums the 8 partials (the all-reduce / unshard for row-parallel TP).

All matmul operands fp16 (PE full rate; fp32 is 1/4 rate), fp32 PSUM
accumulation. Layouts keep the pipeline transpose-free except one 128x128
PE transpose per output tile (O -> O^T for the projection). Softmax sums
come free as a ones-column appended to V; normalization is applied to O
before projection. exp runs on the scalar engine over paired 2-bank PSUM
tiles; diagonal tiles are restricted to their valid causal column range.
"""

import sys

for _p in ("/opt/trn_rl_repo",):
    if _p not in sys.path:
        sys.path.append(_p)

import numpy as np

# ---- problem constants (hardcoded per the task contract) ----
B, T, C, H = 2, 2048, 2048, 16
D = C // H  # 128
NCORES = 8
HPC = H // NCORES  # heads per core = 2
NTOK = B * T  # 4096
P = 128
CT = C // P  # 16 contraction tiles
TOK512 = NTOK // 512  # 8
NQ = T // 512  # q-tiles per unit = 4
TT = NTOK // P  # 32 token 128-tiles
SCALE = 1.0 / np.sqrt(D)

_compiled = None

# tuning knobs (overridable before _build_bass for experiments)
KNOBS = {"cs": 2, "co": 4, "work": 2, "pt": 6, "ysb": 8, "rope": 4, "osb": 8, "xt": 2, "dact": 2, "oact": 1, "otact": 1}


def _build_bass():
    import concourse.bacc as bacc
    import concourse.mybir as mybir
    import concourse.tile as tile
    from contextlib import ExitStack

    f16 = mybir.dt.float16
    f32 = mybir.dt.float32
    Exp = mybir.ActivationFunctionType.Exp

    nc = bacc.Bacc()

    xT = nc.declare_dram_parameter("xT", [C, NTOK], f16, isOutput=False)
    wqk = nc.declare_dram_parameter("wqk", [C, 2 * HPC * D], f16, isOutput=False)
    wv = nc.declare_dram_parameter("wv", [C, HPC * D], f16, isOutput=False)
    wproj = nc.declare_dram_parameter("wproj", [HPC * D, C], f16, isOutput=False)
    cos_t = nc.declare_dram_parameter("cos_t", [P, NTOK], f16, isOutput=False)
    sin_t = nc.declare_dram_parameter("sin_t", [P, NTOK], f16, isOutput=False)
    maskw = nc.declare_dram_parameter("maskw", [P, 1024], f16, isOutput=False)
    ident = nc.declare_dram_parameter("ident", [P, P], f16, isOutput=False)
    rotmp = nc.declare_dram_parameter("rotm", [P, P], f16, isOutput=False)
    y = nc.declare_dram_parameter("y", [NTOK, C], f16, isOutput=True)

    with tile.TileContext(nc) as tc, ExitStack() as ctx:
        pers = ctx.enter_context(tc.tile_pool(name="pers", bufs=1))

        # ---- persistent SBUF tensors ----
        wqk_sb = pers.tile([P, CT, 4 * P], f16)  # [c128, ct, (q0,q1,k0,k1)*128]
        wv_sb = pers.tile([P, CT, 2 * P], f16)
        wproj_sb = pers.tile([P, HPC, C], f16)
        cos_sb = pers.tile([P, NTOK], f16)
        sin_sb = pers.tile([P, NTOK], f16)
        mask_sb = pers.tile([P, 1024], f16)
        id_sb = pers.tile([P, P], f16)
        rotm_sb = pers.tile([P, P], f16)
        qT_sb = pers.tile([P, HPC, NTOK], f16)  # [d, h, tok] rope'd
        kT_sb = pers.tile([P, HPC, NTOK], f16)
        v_sb = pers.tile([P, TT, HPC, D + 1], f16)  # [tokmod, tt, h, D|ones]
        oT_sb = pers.tile([P, TT, HPC, P], f16)  # [d, tt, h, tokmod]

        # ---- working pools (all open for the whole kernel: the stack
        # allocator must never reuse a released zone — released-zone deps
        # blow past the 1-wait/instruction HW limit pre-bacc-split) ----
        xt_pool = ctx.enter_context(tc.tile_pool(name="xt", bufs=KNOBS["xt"]))
        rope_pool = ctx.enter_context(tc.tile_pool(name="rope", bufs=KNOBS["rope"]))
        p_pool = ctx.enter_context(tc.tile_pool(name="pt", bufs=KNOBS["pt"]))
        osb_pool = ctx.enter_context(tc.tile_pool(name="osb", bufs=KNOBS["osb"]))
        ysb_pool = ctx.enter_context(tc.tile_pool(name="ysb", bufs=KNOBS["ysb"]))
        # PSUM (8 banks), phase-dedicated to avoid cross-phase slot stalls:
        #   work: 1-bank x3 (B: qk/rot/v chains; D: yps)
        #   cs:   1-bank x3 (C: S-tiles + transposes) — 3 slots decouple the
        #         PE from exp latency by one extra stage
        #   co:   1-bank x2 (C: packed O accumulators, 2 subs each)
        work_pool = ctx.enter_context(tc.tile_pool(name="work", bufs=KNOBS["work"], space="PSUM"))
        cs_pool = ctx.enter_context(tc.tile_pool(name="cs", bufs=KNOBS["cs"], space="PSUM"))
        co_pool = ctx.enter_context(tc.tile_pool(name="co", bufs=KNOBS["co"], space="PSUM"))

        for cth in range(2):
            nc.sync.dma_start(
                wqk_sb[:, cth * 8 : (cth + 1) * 8, :],
                wqk[cth * 1024 : (cth + 1) * 1024, :].rearrange(
                    "(ct p) m -> p ct m", p=P
                ),
            )
        nc.sync.dma_start(wv_sb[:], wv.rearrange("(ct p) m -> p ct m", p=P))
        nc.sync.dma_start(mask_sb[:], maskw[:])
        nc.sync.dma_start(id_sb[:], ident[:])
        nc.sync.dma_start(rotm_sb[:], rotmp[:])
        nc.vector.memset(v_sb[:, :, :, D : D + 1], 1.0)

        # ======== phase B: qkv projection + rope ========
        for ti in range(TOK512):
            t0 = ti * 512
            xt = xt_pool.tile([P, CT, 512], f16, tag="xt")
            for ch in range(4):
                nc.sync.dma_start(
                    xt[:, ch * 4 : (ch + 1) * 4, :],
                    xT[ch * 512 : (ch + 1) * 512, t0 : t0 + 512].rearrange(
                        "(ct p) j -> p ct j", p=P
                    ),
                )
            # stream rope tables alongside
            nc.sync.dma_start(cos_sb[:, t0 : t0 + 512], cos_t[:, t0 : t0 + 512])
            nc.sync.dma_start(sin_sb[:, t0 : t0 + 512], sin_t[:, t0 : t0 + 512])
            # q,k columns: out^T orientation -> [col128, tok512]
            for ci in range(4):
                hh = ci % HPC
                dstT = qT_sb if ci < HPC else kT_sb
                ps = work_pool.tile([P, 512], f32, tag="work", name="psqk")
                for ct in range(CT):
                    nc.tensor.matmul(
                        ps[:],
                        wqk_sb[:, ct, ci * P : (ci + 1) * P],
                        xt[:, ct, :],
                        start=(ct == 0),
                        stop=(ct == CT - 1),
                    )
                st = rope_pool.tile([P, 512], f16, tag="st")
                nc.vector.tensor_copy(st[:], ps[:])
                # half-rotation via PE permutation matmul (no partition-
                # crossing DVE/DMA needed)
                ps2 = work_pool.tile([P, 512], f32, tag="work", name="psrot")
                nc.tensor.matmul(ps2[:], rotm_sb[:], st[:], start=True, stop=True)
                t1 = rope_pool.tile([P, 512], f16, tag="t1")
                t2 = rope_pool.tile([P, 512], f16, tag="t2")
                nc.vector.tensor_mul(t1[:], st[:], cos_sb[:, t0 : t0 + 512])
                nc.vector.tensor_mul(t2[:], ps2[:], sin_sb[:, t0 : t0 + 512])
                nc.vector.tensor_add(dstT[:, hh, t0 : t0 + 512], t1[:], t2[:])
            # v: natural [tok, D*2] orientation (lhsT = xT tile)
            for sub in range(4):
                vps = work_pool.tile([P, 2 * P], f32, tag="work", name="vps")
                for ct in range(CT):
                    nc.tensor.matmul(
                        vps[:],
                        xt[:, ct, sub * P : (sub + 1) * P],
                        wv_sb[:, ct, :],
                        start=(ct == 0),
                        stop=(ct == CT - 1),
                    )
                tt = ti * 4 + sub
                for h in range(HPC):
                    nc.vector.tensor_copy(
                        v_sb[:, tt, h, 0:D], vps[:, h * P : (h + 1) * P]
                    )

        # ======== phase C: causal attention per (b, h) unit ========
        # S^T per k-tile: [k128, q512]. Diagonal k-tiles restricted to the
        # valid causal column range [g, 512). O accumulators packed 2 subs
        # per 1-bank tile (cols 0 and 256).
        for b in range(B):
            for h in range(HPC):
                toff = b * T
                for qi in range(NQ):
                    q0 = toff + qi * 512
                    ndiag0 = qi * 4  # first diagonal kt
                    nkt = ndiag0 + 4
                    # one accumulator per PSUM bank: two interleaved
                    # accumulation groups sharing a bank lose terms on HW
                    o_tiles = [
                        co_pool.tile([P, D + 1], f32, tag="co", name=f"o{_s}")
                        for _s in range(4)
                    ]

                    def pv(pt_ap, kt, sub_lo):
                        for s in range(sub_lo, 4):
                            nc.tensor.matmul(
                                o_tiles[s][:],
                                pt_ap(s),
                                v_sb[:, b * 16 + kt, h, :],
                                start=(kt == 0),
                                stop=(kt == ndiag0 + s),
                            )

                    for kt in range(nkt):
                        k0 = toff + kt * P
                        gi = kt - ndiag0
                        g = max(gi, 0) * P
                        w = 512 - g
                        sd = cs_pool.tile([P, 512], f32, tag="cs", name="sd")
                        nc.tensor.matmul(
                            sd[:, 0:w],
                            kT_sb[:, h, k0 : k0 + P],
                            qT_sb[:, h, q0 + g : q0 + 512],
                            start=True,
                            stop=True,
                        )
                        ptd = p_pool.tile([P, 512], f16, tag="pt", name="ptd")
                        nc.scalar.activation(
                            ptd[:, 0:w], sd[:, 0:w], Exp, scale=float(SCALE)
                        )
                        if gi >= 0:  # diagonal: multiplicative causal mask
                            nc.vector.tensor_mul(
                                ptd[:, 0:w], ptd[:, 0:w], mask_sb[:, 384 : 384 + w]
                            )
                        pv(
                            lambda s, _g=g: ptd[:, s * P - _g : s * P - _g + P],
                            kt,
                            max(gi, 0),
                        )
                    # drain: normalize O rows by 1/rowsum, transpose to O^T
                    for sub in range(4):
                        tt = b * 16 + qi * 4 + sub
                        ot = o_tiles[sub]
                        rtmp = osb_pool.tile([P, 1], f32, tag="rtmp")
                        nc.vector.reciprocal(rtmp[:], ot[:, D : D + 1])
                        o_sb = osb_pool.tile([P, P], f16, tag="osb")
                        if KNOBS["oact"] and sub % 2 == 1:
                            nc.scalar.mul(o_sb[:], ot[:, 0:D], rtmp[:])
                        else:
                            nc.vector.tensor_scalar_mul(o_sb[:], ot[:, 0:D], rtmp[:])
                        tp = cs_pool.tile([P, P], f16, tag="cs", name="tp")
                        nc.tensor.transpose(tp[:], o_sb[:], id_sb[:])
                        if KNOBS["otact"] and sub % 2 == 0:
                            nc.scalar.copy(oT_sb[:, tt, h, :], tp[:])
                        else:
                            nc.vector.tensor_copy(oT_sb[:, tt, h, :], tp[:])

        # deferred wproj load (only needed for phase D)
        nc.sync.dma_start(wproj_sb[:], wproj.rearrange("(h p) m -> p h m", p=P))

        # deferred wproj load (only needed for phase D)
        nc.sync.dma_start(wproj_sb[:], wproj.rearrange("(h p) m -> p h m", p=P))

        # ======== phase D: output projection ========
        for tt in range(TT):
            for cc in range(4):
                yps = work_pool.tile([P, 512], f32, tag="work", name="yps")
                for h in range(HPC):
                    nc.tensor.matmul(
                        yps[:],
                        oT_sb[:, tt, h, :],
                        wproj_sb[:, h, cc * 512 : (cc + 1) * 512],
                        start=(h == 0),
                        stop=(h == HPC - 1),
                    )
                ysb = ysb_pool.tile([P, 512], f16, tag="ysb")
                if cc % KNOBS["dact"] == 0:
                    nc.scalar.copy(ysb[:], yps[:])
                else:
                    nc.vector.tensor_copy(ysb[:], yps[:])
                nc.sync.dma_start(
                    y[tt * P : (tt + 1) * P, cc * 512 : (cc + 1) * 512], ysb[:]
                )

    # bacc lowering: splits multi-sem waits into EventSemaphore insts
    # (TRN2 allows at most 1 wait per regular instruction), reg alloc, DCE.
    nc.compile()
    return nc


def _host_inputs(x, Wqkv, Wproj):
    """Build per-core device input maps (host-side sharding)."""
    xTf = np.ascontiguousarray(x.reshape(NTOK, C).T).astype(np.float16)

    invf = 1.0 / (10000.0 ** (np.arange(0, D, 2, dtype=np.float32) / D))
    freqs = np.arange(T, dtype=np.float32)[:, None] * invf[None, :]  # [T, 64]
    cos = np.cos(freqs).astype(np.float32).T  # [64, T]
    sin = np.sin(freqs).astype(np.float32).T
    cos_t = np.tile(np.concatenate([cos, cos], axis=0), (1, B)).astype(np.float16)
    sin_t = np.tile(np.concatenate([-sin, sin], axis=0), (1, B)).astype(np.float16)

    ii = np.arange(P)[:, None]
    mm = np.arange(1024)[None, :]
    maskw = (mm >= ii + 384).astype(np.float16)
    ident = np.eye(P, dtype=np.float16)
    rotm = np.zeros((P, P), dtype=np.float16)
    rotm[(np.arange(P) + 64) % P, np.arange(P)] = 1.0

    in_maps = []
    for c in range(NCORES):
        h0 = c * HPC * D  # col offset of this core's heads
        wqk_c = np.concatenate(
            [Wqkv[:, h0 : h0 + HPC * D], Wqkv[:, C + h0 : C + h0 + HPC * D]], axis=1
        ).astype(np.float16)
        wv_c = Wqkv[:, 2 * C + h0 : 2 * C + h0 + HPC * D].astype(np.float16)
        wproj_c = np.ascontiguousarray(Wproj[h0 : h0 + HPC * D, :]).astype(np.float16)
        in_maps.append(
            {
                "xT": xTf,
                "wqk": np.ascontiguousarray(wqk_c),
                "wv": np.ascontiguousarray(wv_c),
                "wproj": wproj_c,
                "cos_t": cos_t,
                "sin_t": sin_t,
                "maskw": maskw,
                "ident": ident,
                "rotm": rotm,
            }
        )
    return in_maps


def kernel(x, Wqkv, Wproj, _trace=False):
    global _compiled
    x = np.asarray(x, dtype=np.float32)
    Wqkv = np.asarray(Wqkv, dtype=np.float32)
    Wproj = np.asarray(Wproj, dtype=np.float32)

    from concourse.bass_utils import run_bass_kernel_spmd

    if _compiled is None:
        _compiled = _build_bass()
    nc = _compiled

    in_maps = _host_inputs(x, Wqkv, Wproj)
    res = run_bass_kernel_spmd(nc, in_maps, list(range(NCORES)), trace=_trace)
    out = np.zeros((NTOK, C), dtype=np.float32)
    for r in res.results:
        out += r["y"].astype(np.float32)
    kernel._last_result = res
    return out.reshape(B, T, C)



# revision 15
# speedup vs baseline: 1.0681x; 1.0681x over previous
"""Causal self-attention (B=2, T=2048, C=2048, H=16, rope) on 8 trn2 cores.

Sharding: tensor-parallel over heads. Each core owns 2 of 16 heads:
  - Wqkv columns for its heads (q,k,v), Wproj rows for its heads.
  - Computes qkv projection, rope, causal attention, and its partial
    output projection y_c = O_c @ Wproj_c  (full [4096, 2048]).
  - Host sums the 8 partials (the all-reduce / unshard for row-parallel TP).

All matmul operands fp16 (PE full rate), fp32 PSUM accumulation.

Structure (v3):
  - PSUM: 8 bank-granular slots in per-tag rings:
      qk x2 (B: qkv-proj accumulators; D: yps even tiles)
      misc x2 (B: rot + v accumulators; D: yps odd tiles)
      sd x2 (C: S^T tiles), o x2 (C: PV accumulators, two-pass)
  - Attention units for batch 0 are emitted interleaved into phase B's
    second half so the scalar engine's exp stream (the phase-C pacer)
    starts ~90us early and phase C work hides under phase-B matmuls.
  - PV runs in two passes per (b,h,q-tile): subs {0,1} inline with the
    S/exp loop, subs {2,3} afterwards from SBUF-cached exp(S) tiles
    (PSUM zero-region rule: one accumulation group per bank). Each sub
    drains (normalize + XBAR dma transpose into O^T) as soon as its
    accumulation stops, to free its bank early.
  - softmax row-sums come free as a ones-column appended to V.
  - Phase D evictions split in 256-col halves over DVE/Pool(/Act when
    free); y rows stored with one DMA per 128-token tile.
"""

import sys

for _p in ("/opt/trn_rl_repo",):
    if _p not in sys.path:
        sys.path.append(_p)

import numpy as np

# ---- problem constants (hardcoded per the task contract) ----
B, T, C, H = 2, 2048, 2048, 16
D = C // H  # 128
NCORES = 8
HPC = H // NCORES  # heads per core = 2
NTOK = B * T  # 4096
P = 128
CT = C // P  # 16 contraction tiles
TOK512 = NTOK // 512  # 8
NQ = T // 512  # q-tiles per unit = 4
TT = NTOK // P  # 32 token 128-tiles
SCALE = 1.0 / np.sqrt(D)

_compiled = None

KNOBS = {"qk": 2, "misc": 2, "sd": 2, "o": 2, "pt": 18, "ysb": 3, "rope": 4, "osb": 8, "xt": 2}


def _build_bass():
    import concourse.bacc as bacc
    import concourse.mybir as mybir
    import concourse.tile as tile
    from contextlib import ExitStack

    f16 = mybir.dt.float16
    f32 = mybir.dt.float32
    Exp = mybir.ActivationFunctionType.Exp

    nc = bacc.Bacc()

    xT = nc.declare_dram_parameter("xT", [C, NTOK], f16, isOutput=False)
    wqk = nc.declare_dram_parameter("wqk", [C, 2 * HPC * D], f16, isOutput=False)
    wv = nc.declare_dram_parameter("wv", [C, HPC * D], f16, isOutput=False)
    wproj = nc.declare_dram_parameter("wproj", [HPC * D, C], f16, isOutput=False)
    cos_t = nc.declare_dram_parameter("cos_t", [P, NTOK], f16, isOutput=False)
    sin_t = nc.declare_dram_parameter("sin_t", [P, NTOK], f16, isOutput=False)
    maskw = nc.declare_dram_parameter("maskw", [P, 1024], f16, isOutput=False)
    rotmp = nc.declare_dram_parameter("rotm", [P, P], f16, isOutput=False)
    y = nc.declare_dram_parameter("y", [NTOK, C], f16, isOutput=True)

    with tile.TileContext(nc) as tc, ExitStack() as ctx:
        pers = ctx.enter_context(tc.tile_pool(name="pers", bufs=1))

        # ---- persistent SBUF tensors ----
        wqk_sb = pers.tile([P, CT, 4 * P], f16)  # [c128, ct, (q0,q1,k0,k1)*128]
        wv_sb = pers.tile([P, CT, 2 * P], f16)
        wproj_sb = pers.tile([P, HPC, C], f16)
        cos_sb = pers.tile([P, NTOK], f16)
        sin_sb = pers.tile([P, NTOK], f16)
        mask_sb = pers.tile([P, 1024], f16)
        rotm_sb = pers.tile([P, P], f16)
        qT_sb = pers.tile([P, HPC, NTOK], f16)  # [d, h, tok] rope'd
        kT_sb = pers.tile([P, HPC, NTOK], f16)
        v_sb = pers.tile([P, TT, HPC, D + 1], f16)  # [tokmod, tt, h, D|ones]
        oT_sb = pers.tile([P, TT, HPC, P], f16)  # [d, tt, h, tokmod]

        # ---- working pools (kept open for the whole kernel) ----
        xt_pool = ctx.enter_context(tc.tile_pool(name="xt", bufs=KNOBS["xt"]))
        rope_pool = ctx.enter_context(tc.tile_pool(name="rope", bufs=KNOBS["rope"]))
        p_pool = ctx.enter_context(tc.tile_pool(name="pt", bufs=KNOBS["pt"]))
        osb_pool = ctx.enter_context(tc.tile_pool(name="osb", bufs=KNOBS["osb"]))
        ysb_pool = ctx.enter_context(tc.tile_pool(name="ysb", bufs=KNOBS["ysb"]))
        # PSUM: bank-granular slots, per-tag rings. 2+2+2+2 = 8 banks.
        qk_pool = ctx.enter_context(tc.tile_pool(name="qkp", bufs=KNOBS["qk"], space="PSUM"))
        misc_pool = ctx.enter_context(tc.tile_pool(name="miscp", bufs=KNOBS["misc"], space="PSUM"))
        sd_pool = ctx.enter_context(tc.tile_pool(name="sdp", bufs=KNOBS["sd"], space="PSUM"))
        o_pool = ctx.enter_context(tc.tile_pool(name="op", bufs=KNOBS["o"], space="PSUM"))

        nc.vector.memset(v_sb[:, :, :, D : D + 1], 1.0)

        # ======== phase B body (qkv projection + rope for one 512-tok tile)
        def gen_b(ti):
            t0 = ti * 512
            xt = xt_pool.tile([P, CT, 512], f16, tag="xt", name="xt")
            if ti == 0:
                # finest-needed-first startup ordering: pair each x chunk
                # with the wqk ct-slices consumed right after it
                for ch in range(4):
                    nc.sync.dma_start(
                        wqk_sb[:, ch * 4 : (ch + 1) * 4, :],
                        wqk[ch * 512 : (ch + 1) * 512, :].rearrange(
                            "(ct p) m -> p ct m", p=P
                        ),
                    )
                    nc.sync.dma_start(
                        xt[:, ch * 4 : (ch + 1) * 4, :],
                        xT[ch * 512 : (ch + 1) * 512, t0 : t0 + 512].rearrange(
                            "(ct p) j -> p ct j", p=P
                        ),
                    )
                    if ch == 0:
                        nc.sync.dma_start(
                            cos_sb[:, t0 : t0 + 512], cos_t[:, t0 : t0 + 512]
                        )
                        nc.sync.dma_start(
                            sin_sb[:, t0 : t0 + 512], sin_t[:, t0 : t0 + 512]
                        )
                        nc.sync.dma_start(rotm_sb[:], rotmp[:])
            else:
                nc.sync.dma_start(
                    xt[:],
                    xT[:, t0 : t0 + 512].rearrange("(ct p) j -> p ct j", p=P),
                )
                nc.sync.dma_start(cos_sb[:, t0 : t0 + 512], cos_t[:, t0 : t0 + 512])
                nc.sync.dma_start(sin_sb[:, t0 : t0 + 512], sin_t[:, t0 : t0 + 512])
            yield
            # q,k columns: out^T orientation -> [col128, tok512]
            for ci in range(4):
                hh = ci % HPC
                dstT = qT_sb if ci < HPC else kT_sb
                ps = qk_pool.tile([P, 512], f32, tag="qk", name="psqk")
                for ct in range(CT):
                    nc.tensor.matmul(
                        ps[:],
                        wqk_sb[:, ct, ci * P : (ci + 1) * P],
                        xt[:, ct, :],
                        start=(ct == 0),
                        stop=(ct == CT - 1),
                    )
                st = rope_pool.tile([P, 512], f16, tag="st")
                nc.vector.tensor_copy(st[:], ps[:])
                # half-rotation via PE permutation matmul
                ps2 = misc_pool.tile([P, 512], f32, tag="misc", name="psrot")
                nc.tensor.matmul(ps2[:], rotm_sb[:], st[:], start=True, stop=True)
                t1 = rope_pool.tile([P, 512], f16, tag="t1")
                t2 = rope_pool.tile([P, 512], f16, tag="t2")
                nc.vector.tensor_mul(t1[:], st[:], cos_sb[:, t0 : t0 + 512])
                nc.vector.tensor_mul(t2[:], ps2[:], sin_sb[:, t0 : t0 + 512])
                nc.vector.tensor_add(dstT[:, hh, t0 : t0 + 512], t1[:], t2[:])
                yield
            if ti == 0:
                nc.sync.dma_start(wv_sb[:], wv.rearrange("(ct p) m -> p ct m", p=P))
                nc.sync.dma_start(mask_sb[:], maskw[:])
            # v: natural [tok, D*2] orientation (lhsT = xT tile)
            for sub in range(4):
                vps = misc_pool.tile([P, 2 * P], f32, tag="misc", name="vps")
                for ct in range(CT):
                    nc.tensor.matmul(
                        vps[:],
                        xt[:, ct, sub * P : (sub + 1) * P],
                        wv_sb[:, ct, :],
                        start=(ct == 0),
                        stop=(ct == CT - 1),
                    )
                tt = ti * 4 + sub
                for h in range(HPC):
                    nc.vector.tensor_copy(
                        v_sb[:, tt, h, 0:D], vps[:, h * P : (h + 1) * P]
                    )
                yield

        # ======== phase C body: causal attention for (b, h, qi) ========
        def gen_c(b, h, qi):
            toff = b * T
            q0 = toff + qi * 512
            ndiag0 = qi * 4  # first diagonal kt
            nkt = ndiag0 + 4

            def drain(o_t, sub):
                tt = b * 16 + qi * 4 + sub
                rtmp = osb_pool.tile([P, 1], f32, tag="rtmp", name="rtmp")
                nc.vector.reciprocal(rtmp[:], o_t[:, D : D + 1])
                o_sb = osb_pool.tile([P, P], f16, tag="osb", name="osb")
                nc.vector.tensor_scalar_mul(o_sb[:], o_t[:, 0:D], rtmp[:])
                nc.sync.dma_start_transpose(oT_sb[:, tt, h, :], o_sb[:])

            ptds = [None] * nkt
            o_lo = [None, None]
            for kt in range(nkt):
                k0 = toff + kt * P
                gi = kt - ndiag0
                g = max(gi, 0) * P
                w = 512 - g
                sd = sd_pool.tile([P, 512], f32, tag="sd", name="sd")
                nc.tensor.matmul(
                    sd[:, 0:w],
                    kT_sb[:, h, k0 : k0 + P],
                    qT_sb[:, h, q0 + g : q0 + 512],
                    start=True,
                    stop=True,
                )
                ptd = p_pool.tile([P, 512], f16, tag="pt", name="ptd")
                nc.scalar.activation(ptd[:, 0:w], sd[:, 0:w], Exp, scale=float(SCALE))
                if gi >= 0:  # diagonal: multiplicative causal mask
                    # on gpsimd: keeps the exp->PV chain off the DVE queue
                    # (DVE head-of-line blocking starves phase-D evictions)
                    meng = nc.gpsimd if KNOBS.get("mask") == "pool" else nc.vector
                    meng.tensor_mul(
                        ptd[:, 0:w], ptd[:, 0:w], mask_sb[:, 384 : 384 + w]
                    )
                ptds[kt] = (ptd, g)
                # pass A: subs 0,1 inline; drain each as soon as it stops
                for s in range(max(gi, 0), 2):
                    if o_lo[s] is None:
                        o_lo[s] = o_pool.tile([P, D + 1], f32, tag="o", name=f"oa{s}")
                    nc.tensor.matmul(
                        o_lo[s][:],
                        ptd[:, s * P - g : s * P - g + P],
                        v_sb[:, b * 16 + kt, h, :],
                        start=(kt == 0),
                        stop=(kt == ndiag0 + s),
                    )
                    if kt == ndiag0 + s:
                        drain(o_lo[s], s)
                yield
            # pass B: subs 2,3 from cached ptd tiles
            o_hi = [None, None]
            for kt in range(nkt):
                gi = kt - ndiag0
                ptd, g = ptds[kt]
                for s in range(max(gi, 2), 4):
                    if o_hi[s - 2] is None:
                        o_hi[s - 2] = o_pool.tile(
                            [P, D + 1], f32, tag="o", name=f"ob{s}"
                        )
                    nc.tensor.matmul(
                        o_hi[s - 2][:],
                        ptd[:, s * P - g : s * P - g + P],
                        v_sb[:, b * 16 + kt, h, :],
                        start=(kt == 0),
                        stop=(kt == ndiag0 + s),
                    )
                    if kt == ndiag0 + s:
                        drain(o_hi[s - 2], s)
                if kt % 2 == 1:
                    yield

        # ======== phase D body: output projection for one 128-token tile
        def gen_d(tt):
            ysb = ysb_pool.tile([P, 4, 512], f16, tag="ysb", name="ysb")
            for cc in range(4):
                pool = qk_pool if cc % 2 == 0 else misc_pool
                tag = "qk" if cc % 2 == 0 else "misc"
                yps = pool.tile([P, 512], f32, tag=tag, name="yps")
                for h in range(HPC):
                    nc.tensor.matmul(
                        yps[:],
                        oT_sb[:, tt, h, :],
                        wproj_sb[:, h, cc * 512 : (cc + 1) * 512],
                        start=(h == 0),
                        stop=(h == HPC - 1),
                    )
                if cc % 2 == 0:
                    nc.vector.tensor_copy(ysb[:, cc, :], yps[:])
                else:
                    nc.scalar.copy(ysb[:, cc, :], yps[:])
                yield
            yeng = nc.gpsimd if KNOBS.get("ydma") == "pool" else nc.sync
            yeng.dma_start(
                y[tt * P : (tt + 1) * P, :],
                ysb[:].rearrange("p c j -> p (c j)"),
            )

        # ======== emission order (order = scheduler priority) ========
        # Attention units are interleaved into phase B's emission as soon
        # as their (q, k) token tiles exist, so the scalar engine's exp
        # stream (the phase-C pacer) runs throughout phase B instead of
        # piling up at the end. Phase D follows, with the last attention
        # unit interleaved into D(b0).
        def chain(gens):
            for g in gens:
                yield from g

        def interleave(gens):
            gens = list(gens)
            while gens:
                for g in list(gens):
                    try:
                        next(g)
                    except StopIteration:
                        gens.remove(g)

        windows = {
            0: [],
            1: [(0, 0, 0)],
            2: [(0, 0, 1), (0, 1, 0)],
            3: [(0, 0, 2), (0, 1, 1)],
            4: [(0, 0, 3), (0, 1, 2), (1, 0, 0)],
            5: [(0, 1, 3), (1, 0, 1), (1, 1, 0)],
            6: [(1, 0, 2), (1, 1, 1)],
            7: [(1, 0, 3), (1, 1, 2)],
        }
        for ti in range(8):
            cg = chain([gen_c(*u) for u in windows[ti]])
            interleave([gen_b(ti), cg])
            if ti == 5:
                # wproj load lands during attention (needed in phase D)
                nc.sync.dma_start(
                    wproj_sb[:], wproj.rearrange("(h p) m -> p h m", p=P)
                )
        interleave([chain([gen_d(tt) for tt in range(16)]), gen_c(1, 1, 3)])
        for tt in range(16, TT):
            for _ in gen_d(tt):
                pass

    nc.compile()
    return nc


def _host_inputs(x, Wqkv, Wproj):
    """Build per-core device input maps (host-side sharding)."""
    xTf = np.ascontiguousarray(x.reshape(NTOK, C).T).astype(np.float16)

    invf = 1.0 / (10000.0 ** (np.arange(0, D, 2, dtype=np.float32) / D))
    freqs = np.arange(T, dtype=np.float32)[:, None] * invf[None, :]  # [T, 64]
    cos = np.cos(freqs).astype(np.float32).T  # [64, T]
    sin = np.sin(freqs).astype(np.float32).T
    cos_t = np.tile(np.concatenate([cos, cos], axis=0), (1, B)).astype(np.float16)
    sin_t = np.tile(np.concatenate([-sin, sin], axis=0), (1, B)).astype(np.float16)

    ii = np.arange(P)[:, None]
    mm = np.arange(1024)[None, :]
    maskw = (mm >= ii + 384).astype(np.float16)
    rotm = np.zeros((P, P), dtype=np.float16)
    rotm[(np.arange(P) + 64) % P, np.arange(P)] = 1.0

    in_maps = []
    for c in range(NCORES):
        h0 = c * HPC * D  # col offset of this core's heads
        wqk_c = np.concatenate(
            [Wqkv[:, h0 : h0 + HPC * D], Wqkv[:, C + h0 : C + h0 + HPC * D]], axis=1
        ).astype(np.float16)
        wv_c = Wqkv[:, 2 * C + h0 : 2 * C + h0 + HPC * D].astype(np.float16)
        wproj_c = np.ascontiguousarray(Wproj[h0 : h0 + HPC * D, :]).astype(np.float16)
        in_maps.append(
            {
                "xT": xTf,
                "wqk": np.ascontiguousarray(wqk_c),
                "wv": np.ascontiguousarray(wv_c),
                "wproj": wproj_c,
                "cos_t": cos_t,
                "sin_t": sin_t,
                "maskw": maskw,
                "rotm": rotm,
            }
        )
    return in_maps


def kernel(x, Wqkv, Wproj, _trace=False):
    global _compiled
    x = np.asarray(x, dtype=np.float32)
    Wqkv = np.asarray(Wqkv, dtype=np.float32)
    Wproj = np.asarray(Wproj, dtype=np.float32)

    from concourse.bass_utils import run_bass_kernel_spmd

    if _compiled is None:
        _compiled = _build_bass()
    nc = _compiled

    in_maps = _host_inputs(x, Wqkv, Wproj)
    res = run_bass_kernel_spmd(nc, in_maps, list(range(NCORES)), trace=_trace)
    out = np.zeros((NTOK, C), dtype=np.float32)
    for r in res.results:
        out += r["y"].astype(np.float32)
    kernel._last_result = res
    return out.reshape(B, T, C)
